# revision 19
# baseline (speedup 1.0000x reference)
"""Causal self-attention (B=2, T=2048, C=1024, H=16) on 8 TRN2 NeuronCores.

Sharding: core c -> batch b = c//4, head-group g = c%4 (4 heads each).
  - qkv: column-sharded per head group; x passed pre-transposed (and bf16) per batch.
  - attention: flash-style, S^T layout (keys on partition), no max-subtraction
    (scores ~ N(0,1), exp is safe in fp32), denominator via ones-column in V.
    Query chunks of 512; key-chunk pairs share one [128,1024] PSUM tile so each
    exp ACTIVATE covers 2 blocks.
  - normalization (1/denominator) is delayed one query-chunk behind the
    score/AV matmuls so the PE never stalls on the Scalar ln/exp chain.
  - proj: Megatron ROW-sharded — each core computes the full [T, 1024] partial
    projection of its own 4 heads (y_local @ W_proj[local rows]); the sum over
    head groups happens on the HOST during unsharding (fp16 partials), so no
    device collective is needed at all.
Matmul inputs are bf16 (1 cycle/row on the PE); accumulation is fp32 in PSUM.
"""

import sys

sys.path.insert(0, "/opt/trn_rl_repo")

import ml_dtypes
import numpy as np

import concourse.bass as bass
import concourse.mybir as mybir
import concourse.tile as tile
from concourse.bass_utils import run_bass_kernel_spmd

B, T, C, H = 2, 2048, 1024, 16
HD = C // H  # 64
HG = 4  # heads per core
CG = HG * HD  # 256 features per core
TQ = 512  # query chunk
TK = 128  # key chunk
NQC = T // TQ  # 4
NCC = C // 128  # 8 contraction chunks
SCALE = 1.0 / np.sqrt(HD)

F32 = mybir.dt.float32
F16 = mybir.dt.float16
BF16 = mybir.dt.bfloat16
BF = ml_dtypes.bfloat16


def _mask_np():
    # diag mask [128, 512] = [M0 | M1]: Md[k, q'] = 1 if k + 128*d <= q',
    # q' in [0, 256) — the two triangular patterns the split diagonal needs.
    k = np.arange(TK)[:, None]
    q = np.arange(TQ // 2)[None, :]
    blocks = [(k + TK * d <= q).astype(np.float32) for d in range(2)]
    return np.concatenate(blocks, axis=1).astype(BF)  # [128, 512]


# Walrus in this image accepts at most ONE semaphore wait per engine
# instruction (the 64B ISA structs have a single EVENTS slot). Tile emits
# multi-wait instructions; hoist the extras onto standalone EventSemaphore
# instructions right before each offender (same engine => same issue order).
_LEGALIZE_SKIP = {
    "InstEventSemaphore",
    "InstCollectiveCompute",
    "InstUnconditionalBranch",
    "InstConditionalBranch",
    "InstRegisterMove",
    "InstCall",
    "InstISA",
}


def _legalize_sync_waits(nc):
    n = 0
    for bb in nc.main_func.blocks:
        insts = bb.instructions
        k = 0
        while k < len(insts):
            inst = insts[k]
            si = inst.sync_info
            ws = list(si.on_wait) if si and si.on_wait else []
            if type(inst).__name__ not in _LEGALIZE_SKIP and len(ws) > 1:
                for w in ws[:-1]:
                    n += 1
                    ev = mybir.InstEventSemaphore(
                        name=f"xwait_{n}", engine=inst.engine
                    )
                    ev.sync_info = mybir.SyncInfo(on_wait=[w], on_update=[])
                    nc.register_instruction(ev)
                    insts.insert(k, ev)
                    k += 1
                inst.sync_info = mybir.SyncInfo(
                    on_wait=[ws[-1]], on_update=list(si.on_update or [])
                )
            k += 1
    return n


def _build_program():
    nc = bass.Bass()

    xT = nc.declare_dram_parameter("xT", [C, T], BF16, isOutput=False)
    w_qk = nc.declare_dram_parameter("w_qk", [C, 2 * CG], BF16, isOutput=False)
    w_v = nc.declare_dram_parameter("w_v", [C, HG * (HD + 1)], BF16, isOutput=False)
    # local proj rows only: [256, 1024] (row-sharded Megatron proj)
    w_pr = nc.declare_dram_parameter("w_pr", [CG, C], BF16, isOutput=False)
    # partial projection output for this head group, fp16; host sums groups
    outP = nc.declare_dram_parameter("outP", [T, C], F16, isOutput=True)

    mask_dram = nc.inline_tensor(_mask_np(), name="masks")

    with tile.TileContext(nc) as tc:
        with (
            tc.tile_pool(name="xtp", bufs=8) as xt_pool,
            tc.tile_pool(name="wqk", bufs=8) as wqk_pool,
            tc.tile_pool(name="wsm", bufs=8) as wsm_pool,
            tc.tile_pool(name="qkT", bufs=4) as qkT_pool,
            tc.tile_pool(name="vp", bufs=16) as vp_pool,
            tc.tile_pool(name="ysb", bufs=2) as y_pool,
            tc.tile_pool(name="ptile", bufs=6) as p_pool,
            tc.tile_pool(name="small", bufs=8) as small_pool,
            tc.tile_pool(name="psS", bufs=2, space="PSUM") as psS_pool,
            tc.tile_pool(name="psY", bufs=2, space="PSUM") as psY_pool,
            tc.tile_pool(name="psA", bufs=2, space="PSUM") as psA_pool,
        ):
            # ---- input tiles; loads emitted in priority order ----
            xT_sb = [
                xt_pool.tile([128, T], BF16, tag="xt", name=f"xT{cc}")
                for cc in range(NCC)
            ]
            _ENG3 = [nc.sync, nc.scalar, nc.gpsimd]
            _eng_i = 0

            def _eng():
                nonlocal _eng_i
                e = _ENG3[_eng_i % 3]
                _eng_i += 1
                return e

            def _load_x(nj):
                for cc in range(NCC):
                    _eng().dma_start(
                        xT_sb[cc][:, nj * 512 : (nj + 1) * 512],
                        xT[cc * 128 : (cc + 1) * 128, nj * 512 : (nj + 1) * 512],
                    )

            # priority 1: w_qk + xT nj0 interleaved across all three queues
            w_qk_sb = []
            for cc in range(NCC):
                t_w = wqk_pool.tile([128, 2 * CG], BF16, tag="wqk", name=f"wqk{cc}")
                _eng().dma_start(t_w[:], w_qk[cc * 128 : (cc + 1) * 128, :])
                w_qk_sb.append(t_w)
            _load_x(0)
            # priority 2: w_v (needed by early v-gen) + xT nj1
            w_v_sb = []
            for cc in range(NCC):
                t_v = wsm_pool.tile([128, HG * (HD + 1)], BF16, tag="wv", name=f"wv{cc}")
                _eng().dma_start(t_v[:], w_v[cc * 128 : (cc + 1) * 128, :])
                w_v_sb.append(t_v)
            _load_x(1)
            mask_sb = small_pool.tile([128, TQ], BF16, tag="mask", name="mask_sb")
            nc.gpsimd.dma_start(mask_sb[:], mask_dram[:, :])
            # priority 3: remaining x columns, then proj weights
            _load_x(2)
            _load_x(3)
            # w_pr: 2 tiles [128, 1024] (chunk hp = rows of head-pair hp)
            w_pr_sb = []
            for hp in range(2):
                t_p = wsm_pool.tile([128, C], BF16, tag="wpr", name=f"wpr{hp}")
                nc.gpsimd.dma_start(t_p[:], w_pr[hp * 128 : (hp + 1) * 128, :])
                w_pr_sb.append(t_p)
            ones64 = small_pool.tile([1, 64], BF16, tag="ones64", name="ones64")
            nc.vector.memset(ones64[:], 1.0)

            # ---- qk^T = (x @ w_qk)^T tiles [128, T] bf16 ----
            # mi 0: q heads 0-1, mi 1: q heads 2-3, mi 2: k heads 0-1, mi 3: k heads 2-3
            qkT_sb = [None] * 4

            def emit_qkT(mi, njs):
                if qkT_sb[mi] is None:
                    qkT_sb[mi] = qkT_pool.tile([128, T], BF16, tag="qkT", name=f"qkT{mi}")
                t_qk = qkT_sb[mi]
                for nj in njs:
                    ps = psA_pool.tile([128, 512], F32, tag="psA")
                    for cc in range(NCC):
                        nc.tensor.matmul(
                            ps[:],
                            lhsT=w_qk_sb[cc][:, mi * 128 : (mi + 1) * 128],
                            rhs=xT_sb[cc][:, nj * 512 : (nj + 1) * 512],
                            start=(cc == 0),
                            stop=(cc == NCC - 1),
                        )
                    nc.vector.tensor_copy(t_qk[:, nj * 512 : (nj + 1) * 512], ps[:])

            def qT(h):  # [64, T] view, queries of head h, transposed
                return qkT_sb[h // 2][64 * (h % 2) : 64 * (h % 2) + 64, :]

            def kT(h):
                return qkT_sb[2 + h // 2][64 * (h % 2) : 64 * (h % 2) + 64, :]

            # ---- v' tiles: [128, 4*65] bf16, per head [v_h | 1] ----
            vp_sb = [None] * (T // TK)

            def emit_v(tis):
                for ti in tis:
                    ps = psA_pool.tile([128, HG * (HD + 1)], F32, tag="psA")
                    for cc in range(NCC):
                        nc.tensor.matmul(
                            ps[:],
                            lhsT=xT_sb[cc][:, ti * 128 : (ti + 1) * 128],
                            rhs=w_v_sb[cc][:],
                            start=(cc == 0),
                            stop=(cc == NCC - 1),
                        )
                    t_vp = vp_pool.tile(
                        [128, HG * (HD + 1)], BF16, tag="vp", name=f"vp{ti}"
                    )
                    # w_v has a zero column per head; overwrite those with ones
                    nc.vector.tensor_copy(t_vp[:], ps[:])
                    for h in range(HG):
                        nc.vector.memset(t_vp[:, h * 65 + 64 : h * 65 + 65], 1.0)
                    vp_sb[ti] = t_vp

            # ---- attention core for one (head, query-chunk) ----
            y_sb = [
                y_pool.tile([128, T], BF16, tag="ysb", name=f"ysb{i}") for i in range(2)
            ]
            yf_sb = {}

            def att_core(h, qc):
                HQ = TQ // 2  # 256
                q0 = qc * TQ
                b0 = 4 * qc  # first diagonal key chunk
                ps_y = psY_pool.tile([65, TQ], F32, tag="psY")

                def vp_h(kc):
                    return vp_sb[kc][:, h * 65 : (h + 1) * 65]

                # full key-pair blocks below the diagonal (no mask)
                for j in range(2 * qc):
                    ps_s = psS_pool.tile([128, 2 * TQ], F32, tag="psS")
                    for half in range(2):
                        kc = 2 * j + half
                        nc.tensor.matmul(
                            ps_s[:, half * TQ : (half + 1) * TQ],
                            lhsT=kT(h)[:, kc * TK : (kc + 1) * TK],
                            rhs=qT(h)[:, q0 : q0 + TQ],
                            start=True,
                            stop=True,
                        )
                    p_t = p_pool.tile([128, 2 * TQ], BF16, tag="ptile")
                    nc.scalar.activation(
                        p_t[:],
                        ps_s[:],
                        mybir.ActivationFunctionType.Exp,
                        scale=float(SCALE),
                    )
                    for half in range(2):
                        kc = 2 * j + half
                        nc.tensor.matmul(
                            ps_y[:],
                            lhsT=vp_h(kc),
                            rhs=p_t[:, half * TQ : (half + 1) * TQ],
                            start=(kc == 0),
                            stop=False,
                        )
                # diagonal 512x512 block, query-split in halves of 256:
                #  qh0 needs key chunks b0+0..1, qh1 needs b0+0..3 ->
                #  tile A packs [d0|d1]x qh0 , [d0|d1] x qh1 ; tile B [d2|d3] x qh1
                ps_a = psS_pool.tile([128, 2 * TQ], F32, tag="psS")
                for d in range(2):
                    kc = b0 + d
                    nc.tensor.matmul(
                        ps_a[:, d * HQ : (d + 1) * HQ],
                        lhsT=kT(h)[:, kc * TK : (kc + 1) * TK],
                        rhs=qT(h)[:, q0 : q0 + HQ],
                        start=True,
                        stop=True,
                    )
                    nc.tensor.matmul(
                        ps_a[:, TQ + d * HQ : TQ + (d + 1) * HQ],
                        lhsT=kT(h)[:, kc * TK : (kc + 1) * TK],
                        rhs=qT(h)[:, q0 + HQ : q0 + TQ],
                        start=True,
                        stop=True,
                    )
                ps_b2 = psA_pool.tile([128, TQ], F32, tag="psA")
                for d in range(2, 4):
                    kc = b0 + d
                    nc.tensor.matmul(
                        ps_b2[:, (d - 2) * HQ : (d - 1) * HQ],
                        lhsT=kT(h)[:, kc * TK : (kc + 1) * TK],
                        rhs=qT(h)[:, q0 + HQ : q0 + TQ],
                        start=True,
                        stop=True,
                    )
                p_a = p_pool.tile([128, 2 * TQ], BF16, tag="ptile")
                nc.scalar.activation(
                    p_a[:], ps_a[:], mybir.ActivationFunctionType.Exp,
                    scale=float(SCALE),
                )
                nc.vector.tensor_mul(p_a[:, 0:TQ], p_a[:, 0:TQ], mask_sb[:])
                p_b = p_pool.tile([128, TQ], BF16, tag="ptileB", bufs=4)
                nc.scalar.activation(
                    p_b[:], ps_b2[:], mybir.ActivationFunctionType.Exp,
                    scale=float(SCALE),
                )
                nc.vector.tensor_mul(p_b[:], p_b[:], mask_sb[:])
                first = qc == 0
                for d in range(2):
                    nc.tensor.matmul(
                        ps_y[:, 0:HQ],
                        lhsT=vp_h(b0 + d),
                        rhs=p_a[:, d * HQ : (d + 1) * HQ],
                        start=(first and d == 0),
                        stop=False,
                        skip_group_check=True,
                    )
                for d in range(2):
                    nc.tensor.matmul(
                        ps_y[:, HQ:TQ],
                        lhsT=vp_h(b0 + d),
                        rhs=p_a[:, TQ + d * HQ : TQ + (d + 1) * HQ],
                        start=(first and d == 0),
                        stop=False,
                        skip_group_check=True,
                    )
                for d in range(2, 4):
                    nc.tensor.matmul(
                        ps_y[:, HQ:TQ],
                        lhsT=vp_h(b0 + d),
                        rhs=p_b[:, (d - 2) * HQ : (d - 1) * HQ],
                        start=False,
                        stop=(d == 3),
                        skip_group_check=True,
                    )
                # evacuate PSUM promptly; frees the bank for the next chunk
                yf = small_pool.tile([65, TQ], F32, tag="yf", bufs=4, name=f"yf{h}_{qc}")
                nc.vector.tensor_copy(yf[:], ps_y[:])
                yf_sb[(h, qc)] = yf

            def finish_qc(hp, qc):
                # 1/d = exp(-ln(d)) on ACT (both fns live in the same table
                # set), broadcast across partitions via a ones-column matmul.
                for hh in range(2):
                    h = 2 * hp + hh
                    yf = yf_sb.pop((h, qc))
                    den_ln = small_pool.tile([1, TQ], F32, tag="recipf", bufs=3)
                    nc.scalar.activation(
                        den_ln[:], yf[64:65, :], mybir.ActivationFunctionType.Ln
                    )
                    recip = small_pool.tile([1, TQ], BF16, tag="recip", bufs=3)
                    nc.scalar.activation(
                        recip[:],
                        den_ln[:],
                        mybir.ActivationFunctionType.Exp,
                        scale=-1.0,
                    )
                    ps_b = psA_pool.tile([64, TQ], F32, tag="psA")
                    nc.tensor.matmul(
                        ps_b[:], lhsT=ones64[:], rhs=recip[:], start=True, stop=True
                    )
                    b_sb = small_pool.tile([64, TQ], BF16, tag="bsb", bufs=3)
                    nc.vector.tensor_copy(b_sb[:], ps_b[:])
                    nc.vector.tensor_mul(
                        y_sb[hp][64 * hh : 64 * hh + 64, qc * TQ : (qc + 1) * TQ],
                        yf[0:64, :],
                        b_sb[:],
                    )

            # ---- partial proj for t-blocks of one query chunk ----
            # outP[tb, :] = y_sb[0][:, tb].T @ w_pr[0] + y_sb[1][:, tb].T @ w_pr[1]
            def proj(qc):
                # PSUM evacuation and the output-store trigger both live on the
                # otherwise-idle GpSimd engine: the dma_start's wait on the
                # copy is a same-engine ordering no-op, so neither the Scalar
                # (EXP) nor Vector stream ever stalls on proj stores.
                for tb in range(4 * qc, 4 * qc + 4):
                    o_t = small_pool.tile([128, C], F16, tag="otile", bufs=3)
                    for ch in range(2):
                        ps = psA_pool.tile([128, 512], F32, tag="psA")
                        for hp in range(2):
                            nc.tensor.matmul(
                                ps[:],
                                lhsT=y_sb[hp][:, tb * 128 : (tb + 1) * 128],
                                rhs=w_pr_sb[hp][:, ch * 512 : (ch + 1) * 512],
                                start=(hp == 0),
                                stop=(hp == 1),
                            )
                        nc.vector.tensor_copy(o_t[:, ch * 512 : (ch + 1) * 512], ps[:])
                    nc.gpsimd.dma_start(outP[tb * 128 : (tb + 1) * 128, :], o_t[:])

            # ---- emission schedule: keep the PE dense, finish lags one chunk ----
            emit_qkT(0, [0])
            emit_qkT(2, [0])
            emit_qkT(0, [1])
            emit_qkT(2, [1])
            emit_v([0, 1, 2, 3])
            att_core(0, 0)
            att_core(1, 0)
            emit_v([4, 5, 6, 7])
            emit_qkT(0, [2])
            emit_qkT(2, [2])
            finish_qc(0, 0)
            att_core(0, 1)
            att_core(1, 1)
            emit_v([8, 9, 10, 11])
            emit_qkT(0, [3])
            emit_qkT(2, [3])
            finish_qc(0, 1)
            att_core(0, 2)
            att_core(1, 2)
            emit_v([12, 13, 14, 15])
            emit_qkT(1, [0, 1])
            finish_qc(0, 2)
            att_core(0, 3)
            emit_qkT(1, [2, 3])
            att_core(1, 3)
            emit_qkT(3, [0, 1])
            finish_qc(0, 3)
            emit_qkT(3, [2, 3])
            att_core(2, 0)
            att_core(3, 0)
            finish_qc(1, 0)
            att_core(2, 1)
            proj(0)
            att_core(3, 1)
            finish_qc(1, 1)
            att_core(2, 2)
            proj(1)
            att_core(3, 2)
            finish_qc(1, 2)
            att_core(2, 3)
            proj(2)
            att_core(3, 3)
            finish_qc(1, 3)
            proj(3)

    _legalize_sync_waits(nc)
    return nc


_NC_CACHE = None


def _get_nc():
    global _NC_CACHE
    if _NC_CACHE is None:
        _NC_CACHE = _build_program()
    return _NC_CACHE


def _shard_inputs(x, w_qkv, w_proj):
    """Per-core input maps (bf16). Core c: batch c//4, head group c%4."""
    x = np.asarray(x, np.float32)
    w_qkv = np.asarray(w_qkv, np.float32)
    w_proj = np.asarray(w_proj, np.float32)
    xT = [np.ascontiguousarray(x[b].T).astype(BF) for b in range(B)]  # [C, T]
    wq = w_qkv[:, 0:C]
    wk = w_qkv[:, C : 2 * C]
    wv = w_qkv[:, 2 * C : 3 * C]
    in_maps = []
    for c in range(8):
        b, g = c // 4, c % 4
        cols = slice(g * CG, (g + 1) * CG)
        in_maps.append(
            {
                "xT": xT[b],
                "w_qk": np.ascontiguousarray(
                    np.concatenate([wq[:, cols], wk[:, cols]], axis=1)
                ).astype(BF),
                "w_v": np.ascontiguousarray(
                    np.concatenate(
                        [
                            np.concatenate(
                                [
                                    wv[:, g * CG + h * HD : g * CG + (h + 1) * HD],
                                    np.zeros((C, 1), np.float32),
                                ],
                                axis=1,
                            )
                            for h in range(HG)
                        ],
                        axis=1,
                    )
                ).astype(BF),
                "w_pr": np.ascontiguousarray(w_proj[g * CG : (g + 1) * CG, :]).astype(
                    BF
                ),
            }
        )
    return in_maps


def _assemble(results):
    out = np.empty((B, T, C), np.float32)
    for b in range(B):
        acc = results[4 * b]["outP"].astype(np.float32)
        for g in range(1, 4):
            acc += results[4 * b + g]["outP"].astype(np.float32)
        out[b] = acc
    return out


def kernel(x, w_qkv, w_proj, **run_kwargs):
    nc = _get_nc()
    in_maps = _shard_inputs(x, w_qkv, w_proj)
    res = run_bass_kernel_spmd(nc, in_maps, core_ids=list(range(8)), **run_kwargs)
    out = _assemble(res.results)
    if run_kwargs:
        return out, res
    return out
